# revision 28
# baseline (speedup 1.0000x reference)
"""Trainium2 Bass kernel for nn_MultiHeadSelfAttention_29403346108551.

Reference semantics (faithful to the original nn.Module):
  q/k/v = (x @ W.T + b) .reshape(b, 16, 2048, 64)   # reshape, NOT transpose
  RoPE with a *scalar* position t=seq_len (same angle for every token),
  scores = q k^T / 8, softmax, o = p v, merge heads, o @ wo.T + bo.

Structural facts used for sharding:
  - The head split is a row-major reshape: head h reads x rows [128h, 128h+128)
    and ALL 1024 features; within-head time t = r*16 + jc (r = x-row in block,
    jc = feature chunk j//64), d = j%64.  Permuted time t'' = jc*128 + r is
    used on-device; the host un-permutes.
  - RoPE rotation folded into wq/wk/bq/bk on the host (scalar position).
  - Core cid: batch cid//4, head group cid%4 (4 heads = x rows [512g, 512g+512)).
    Output projection partials summed across the 4 cores of a batch on host
    (device outputs bf16 partials; host accumulates in f32).

Design (v7) - ACT(exp)-bound pipeline, single unified stream:
  - The softmax exp (128 tiles of [128,1024] on ACT @ ~1.1us each) is the hard
    floor (~143us).  Everything else is scheduled AROUND a never-stalling exp
    stream: first exp at ~9us, last +143us, short tail.
  - Scores matmuls are 2 CONCURRENT row-tiled MMs (contraction d=64/head):
    head A in PE rows 0-63, head B in rows 64-127 (auto tile_position from
    base partitions) -> ~2x scores throughput vs the padded full-array form.
    qT2 overlays qA (partitions 0-63) and qB (64-127) in the same columns.
  - All projections (Q/K/V, both pairs merged, N=512 moving) are emitted as
    dependency-ordered FILLER inside the attention j-loop, paced to fill the
    PE gap under each exp.  Weights arrive from a consumption-ordered DRAM
    blob via 3 parallel DMA rings (scalar ring: critical prefix; gpsimd: wv;
    sync: the rest).
  - Softmax denominators via the ones-column in v_aug (PV stationary [128,65]).
  - Output projection finals are filler too; 2 units + the last group are
    reserved as tail latency-hiding.  Output partials are bf16.
"""

import numpy as np
import ml_dtypes

import concourse.bass as bass
import concourse.mybir as mybir
import concourse.tile as tile
from concourse import bacc
from concourse.bass_utils import run_bass_kernel_spmd

F32 = mybir.dt.float32
F32R = mybir.dt.float32r
BF16 = mybir.dt.bfloat16
I16 = mybir.dt.int16

MODEL_DIM = 1024
NUM_HEADS = 16
D_K = 64
B = 2
T = 2048
N_CORES = 8
NK = 8              # contraction chunks of 128 over MODEL_DIM
RPC = 512           # x rows per core
SEQ_POS = 2048      # scalar rope position used by the reference

# weight blob column offsets (bf16 cols), consumption-ordered segments
WQ_OFF = [0, 1024, 18432, 19456, 20480, 21504, 22528, 23552]
WK_OFF = [2048, 3072, 4096, 5120, 10240, 11264, 12288, 13312]
WV_OFF = [6144, 14336]
BLOB_COLS = 24576

ES_BUFS = 32        # eS ring depth == max pend lag
MM_NS = 213         # ~N=512 warm matmul issue time
PULL_BUDGET = 1280  # per-j PE-filler budget (ns)


def _build_program() -> bass.Bass:
    nc = bacc.Bacc(None, target_bir_lowering=False, debug=False)

    xt_d = nc.dram_tensor("xt", [128, NK, RPC], BF16, kind="ExternalInput")
    wblob = nc.dram_tensor("wblob", [128, BLOB_COLS], BF16, kind="ExternalInput")
    woT = nc.dram_tensor("woT", [2, 128, MODEL_DIM], BF16, kind="ExternalInput")
    bq = nc.dram_tensor("bq", [128, 8], F32, kind="ExternalInput")
    bk = nc.dram_tensor("bk", [128, 8], F32, kind="ExternalInput")
    bv = nc.dram_tensor("bv", [MODEL_DIM], F32, kind="ExternalInput")
    outp = nc.dram_tensor("outp", [T, MODEL_DIM], BF16, kind="ExternalOutput")

    with tile.TileContext(nc) as tc:
        with (
            tc.tile_pool(name="xpool", bufs=1) as xpool,
            tc.tile_pool(name="wpool", bufs=1) as wpool,
            tc.tile_pool(name="cpool", bufs=1) as cpool,
            tc.tile_pool(name="qkpool", bufs=1) as qkpool,
            tc.tile_pool(name="vpool", bufs=4) as vpool,
            tc.tile_pool(name="espool", bufs=ES_BUFS) as espool,
            tc.tile_pool(name="o2pool", bufs=1) as o2pool,
            tc.tile_pool(name="outpool", bufs=4) as outpool,
            tc.tile_pool(name="opool", bufs=2) as opool,
            tc.tile_pool(name="rcpool", bufs=2) as rcpool,
            tc.tile_pool(name="rcbig", bufs=2) as rcbig,
            tc.tile_pool(name="psS", bufs=2, space="PSUM") as psS_pool,
            tc.tile_pool(name="psO", bufs=1, space="PSUM") as psO_pool,
            tc.tile_pool(name="psfx", bufs=1, space="PSUM") as psF_pool,
        ):
            # ---- persistent tiles ----
            warm_w = cpool.tile([128, 512], BF16, name="warm_w")
            nc.vector.memset(warm_w, 0.0)
            ones64 = cpool.tile([1, 64], F32R, name="ones64")
            nc.vector.memset(ones64.bitcast(F32), 1.0)

            xtile = xpool.tile([128, NK, RPC], BF16, name="xtile")
            wtile = wpool.tile([128, BLOB_COLS], BF16, name="wtile")
            bq_sb = cpool.tile([128, 8], F32, name="bq_sb")
            bk_sb = cpool.tile([128, 8], F32, name="bk_sb")
            bv_bc = cpool.tile([128, MODEL_DIM], F32, name="bv_bc")
            wo_sb = [cpool.tile([128, MODEL_DIM], BF16, name=f"wo_{m_}")
                     for m_ in range(2)]
            warm2 = cpool.tile([1, 16], F32, name="warm2")
            gwa = cpool.tile([1, 16], F32, name="gwa")
            gwb = cpool.tile([4, 16], F32, name="gwb")

            # qT2: qA in partitions 0-63, qB in 64-127 (same columns);
            #   col = (m*4 + wq)*512 + (jc%4)*128 + r   (t'' = jc*128 + r)
            qT2 = qkpool.tile([128, 4096], BF16, name="qT2")
            # kT2a: pair m at cols [m*T, (m+1)*T); partition = 64*parity + d;
            #   col = t'' = jc*128 + r
            kT2a = qkpool.tile([128, 2 * T], BF16, name="kT2a")
            o2T = [o2pool.tile([128, T], BF16, name=f"o2T_{i}") for i in range(2)]

            # v_aug[bl]: [128 keys(r), 16 chunks(jc), 65]; col 64 = ones
            v_aug = []
            for bl in range(4):
                va = vpool.tile([128, 16, 65], BF16, tag="va", name=f"v_aug_{bl}")
                nc.vector.memset(va[:, :, 64:65], 1.0)
                v_aug.append(va)

            # ---- DMA issue: ONE ring (sync), strict consumption order.
            # Round-robining rings at packet granularity caps each ring at
            # ~1/3 of effective HBM rate; a single FIFO ring gives the
            # critical prefix (x + wq01 + wk0) the full rate. ----
            # ACT exp table preload first (independent of DMA)
            nc.scalar.activation(warm2, warm_w.bitcast(F32)[0:1, 0:16],
                                 mybir.ActivationFunctionType.Exp, scale=1.0)
            nc.sync.dma_start(out=xtile[:, 0:4, :], in_=xt_d[:, 0:4, :])
            nc.sync.dma_start(out=wtile[:, 0:2048], in_=wblob[:, 0:2048])
            nc.sync.dma_start(out=xtile[:, 4:8, :], in_=xt_d[:, 4:8, :])
            nc.sync.dma_start(out=wtile[:, 2048:3072],
                              in_=wblob[:, 2048:3072])
            nc.sync.dma_start(out=wtile[:, 3072:6144], in_=wblob[:, 3072:6144])
            nc.sync.dma_start(out=wtile[:, 10240:14336],
                              in_=wblob[:, 10240:14336])
            nc.sync.dma_start(out=wtile[:, 6144:10240],
                              in_=wblob[:, 6144:10240])
            nc.sync.dma_start(out=wtile[:, 14336:18432],
                              in_=wblob[:, 14336:18432])
            nc.sync.dma_start(out=wtile[:, 18432:24576],
                              in_=wblob[:, 18432:24576])
            nc.sync.dma_start(
                out=bv_bc,
                in_=bass.AP(tensor=bv[:].tensor, offset=bv[:].offset,
                            ap=[[0, 128]] + [list(p) for p in bv[:].ap]))
            for m_ in range(2):
                nc.sync.dma_start(out=wo_sb[m_], in_=woT[m_, :, :])
            # gpsimd (SWDGE ring): tiny biases + ucode lib warm only
            nc.gpsimd.dma_start(out=bq_sb, in_=bq[:, :])
            nc.gpsimd.dma_start(out=bk_sb, in_=bk[:, :])
            nc.gpsimd.dma_start(out=gwa, in_=warm_w.bitcast(F32)[0:1, 0:16])
            nc.gpsimd.partition_broadcast(gwb, gwa)
            nc.gpsimd.dma_start(out=gwa, in_=gwb[0:1, :])
            nc.gpsimd.partition_broadcast(gwb, gwa)

            # ---- PE warmup bridging the prefix DMA wait (HAM warm).
            # psO bank: its WAR consumer (wsink) resolves long before PV. ----
            psw = psO_pool.tile([128, 512], F32, tag="oA", name="psw")
            for _ in range(14):
                nc.tensor.matmul(psw, warm_w[:, 0:128], warm_w,
                                 start=True, stop=True)
            wsink = cpool.tile([1, 16], F32, name="wsink")
            nc.vector.tensor_copy(wsink, psw[0:1, 0:16])

            # ---- projection units (merged pairs, N=512) ----
            uid = [0]

            def qk_unit(is_q, p, prefix=False, on_pss=False):
                off = (WQ_OFF if is_q else WK_OFF)[p]
                if on_pss:
                    # prefix K0: a psS bank avoids the flex-tag WAR that
                    # would serialize it behind Q0's drains
                    ps = psS_pool.tile([128, 1024], F32, tag="s", bufs=2,
                                       name=f"psP{p}")[:, 0:512]
                else:
                    ps = psF_pool.tile([128, 512], F32,
                                       tag=f"f{uid[0] % 2}", bufs=1,
                                       name=f"ps{'q' if is_q else 'k'}{p}")
                uid[0] += 1
                for k in range(NK):
                    nc.tensor.matmul(
                        ps, wtile[:, off + k * 128:off + (k + 1) * 128],
                        xtile[:, k, :], start=(k == 0), stop=(k == NK - 1))
                    yield
                bias_sb = bq_sb if is_q else bk_sb
                for half in range(2):
                    jc = 2 * p + half
                    src_h = ps[64 * half:64 * half + 64, :].rearrange(
                        "p (m x) -> p m x", m=2)
                    if is_q:
                        base2 = (jc // 4) * 512 + (jc % 4) * 128
                        dst_t = qT2
                    else:
                        base2 = jc * 128
                        dst_t = kT2a
                    bias_ap = bias_sb[64 * half:64 * half + 64, p:p + 1]
                    for ph in range(2):
                        dst = dst_t[64 * ph:64 * ph + 64, :].rearrange(
                            "p (m z) -> p m z", m=2)[:, :, base2:base2 + 128]
                        src = src_h[:, :, ph * 128:(ph + 1) * 128]
                        if prefix and ph == 0:
                            # ACT is idle pre-exp0: halve the drain chain
                            nc.scalar.activation(
                                dst, src,
                                mybir.ActivationFunctionType.Identity,
                                bias=bias_ap, scale=1.0)
                        else:
                            nc.vector.tensor_scalar_add(dst, src, bias_ap)

            def v_unit(bl, jw):
                off = WV_OFF[jw]
                ps = psF_pool.tile([128, 512], F32, tag=f"f{uid[0] % 2}",
                                   bufs=1, name=f"psv{bl}_{jw}")
                uid[0] += 1
                for k in range(NK):
                    nc.tensor.matmul(
                        ps, xtile[:, k, bl * 128:(bl + 1) * 128],
                        wtile[:, off + k * 512:off + (k + 1) * 512],
                        start=(k == 0), stop=(k == NK - 1))
                    yield
                nc.vector.tensor_tensor(
                    v_aug[bl][:, 8 * jw:8 * jw + 8, 0:64],
                    ps[:, :].rearrange("p (cc d) -> p cc d", d=64),
                    bv_bc[:, jw * 512:(jw + 1) * 512].rearrange(
                        "p (cc d) -> p cc d", d=64),
                    mybir.AluOpType.add)

            # ---- prefix: Q u0, Q u1, K u0 (needed by j=0) ----
            for gen in (qk_unit(True, 0, prefix=True),
                        qk_unit(True, 1, prefix=True),
                        qk_unit(False, 0, prefix=True, on_pss=True)):
                for _ in gen:
                    pass

            # ---- attention machinery ----
            state = {}
            fill_q = []

            def emit_scores(i, wq, m, cc):
                psS = psS_pool.tile([128, 1024], F32, tag="s", bufs=2,
                                    name=f"psS_{i}")
                koff = m * T + cc * 128
                qoff = (m * 4 + wq) * 512
                # High priority keeps the two row-tiled MMs ADJACENT in the
                # PE schedule: back-to-back they run concurrently (disjoint
                # row groups) with LDWEIGHTS hidden; a full-array proj MM
                # between them would serialize all three.
                with tc.high_priority(offset=500000):
                    nc.tensor.matmul(psS[:, 0:512],
                                     kT2a[0:64, koff:koff + 128],
                                     qT2[0:64, qoff:qoff + 512],
                                     start=True, stop=True)
                    nc.tensor.matmul(psS[:, 512:1024],
                                     kT2a[64:128, koff:koff + 128],
                                     qT2[64:128, qoff:qoff + 512],
                                     start=True, stop=True)
                eS = espool.tile([128, 1024], I16, tag="es", bufs=ES_BUFS,
                                 name=f"eS_{i}")
                nc.scalar.activation(
                    eS.bitcast(BF16), psS,
                    mybir.ActivationFunctionType.Exp, scale=0.125)
                return eS

            def normalize_pair(m, wq, psO_A, psO_B, tail_ctx=None):
                """psO_{A,B} [65,512] -> o2T[m][:, wq*512:+512].
                Mid-kernel: gpsimd broadcast (hidden under the PE stream).
                Tail: PE broadcast matmul (gpsimd lib switch costs ~3us)."""
                o_sbs, rcps = [], []
                for ph, psO in ((0, psO_A), (1, psO_B)):
                    o_sb = opool.tile([65, 512], F32, tag=f"osb{ph}",
                                      bufs=2, name=f"osb{ph}_{m}_{wq}")
                    nc.vector.tensor_copy(o_sb, psO)
                    o_sbs.append(o_sb)
                for ph in range(2):
                    den_t = rcpool.tile([128, 4], F32, tag=f"dent{ph}",
                                        bufs=2)
                    nc.gpsimd.dma_start(
                        out=den_t,
                        in_=o_sbs[ph][64:65, :].rearrange(
                            "a (p i) -> a p i", p=128))
                    rcp_t = rcpool.tile([128, 4], F32, tag=f"rcpt{ph}",
                                        bufs=2)
                    nc.vector.reciprocal(rcp_t, den_t)
                    rcps.append(rcp_t)
                flats = []
                for ph in range(2):
                    rcp_flat = rcbig.tile([1, 512], F32, tag=f"rcpf{ph}",
                                          bufs=2)
                    nc.gpsimd.dma_start(
                        out=rcp_flat[0:1, :].rearrange(
                            "a (p i) -> a p i", p=128),
                        in_=rcps[ph])
                    flats.append(rcp_flat)
                for ph in range(2):
                    if tail_ctx is None:
                        rcp_bc = rcbig.tile([64, 512], F32,
                                            tag=f"rcpb{ph}", bufs=2)
                        nc.gpsimd.partition_broadcast(rcp_bc, flats[ph])
                    else:
                        rcp_bc = tail_ctx.tile([64, 512], F32,
                                               tag=f"f{ph}",
                                               name=f"psRc{ph}")
                        nc.tensor.matmul(rcp_bc, ones64,
                                         flats[ph].bitcast(F32R),
                                         start=True, stop=True)
                    nc.vector.tensor_tensor(
                        o2T[m][64 * ph:64 * ph + 64,
                               wq * 512:(wq + 1) * 512],
                        o_sbs[ph][0:64, :], rcp_bc, mybir.AluOpType.mult)

            def finals_gen(tt, tail=False):
                psh = [psF_pool.tile([128, 512], F32, tag=f"f{jj}",
                                     name=f"psF_{tt}_{jj}")
                       for jj in range(2)]
                for jw in range(2):
                    for m2 in range(2):
                        nc.tensor.matmul(
                            psh[jw],
                            o2T[m2][:, tt * 128:(tt + 1) * 128],
                            wo_sb[m2][:, jw * 512:(jw + 1) * 512],
                            start=(m2 == 0), stop=(m2 == 1))
                        yield
                out_sb = outpool.tile([128, MODEL_DIM], BF16,
                                      tag="out", bufs=4, name=f"out_{tt}")
                if tail:
                    # ACT is idle post-exp: splitting the two psum-drain
                    # casts across ACT+DVE halves the tail's cast chain
                    nc.scalar.activation(
                        out_sb[:, 0:512], psh[0],
                        mybir.ActivationFunctionType.Identity, scale=1.0)
                else:
                    nc.vector.tensor_copy(out_sb[:, 0:512], psh[0])
                nc.vector.tensor_copy(out_sb[:, 512:1024], psh[1])
                nc.sync.dma_start(
                    out=outp[tt * 128:(tt + 1) * 128, :], in_=out_sb)

            def emit_pv(i, wq, m, cc, eS):
                if cc == 0:
                    state[(wq, m)] = (
                        psO_pool.tile([65, 512], F32, tag="oA",
                                      name=f"psO_A_{wq}_{m}"),
                        psO_pool.tile([65, 512], F32, tag="oB",
                                      name=f"psO_B_{wq}_{m}"),
                    )
                psO_A, psO_B = state[(wq, m)]
                eSb = eS.bitcast(BF16)
                nc.tensor.matmul(psO_A, v_aug[2 * m][:, cc, :],
                                 eSb[:, 0:512],
                                 start=(cc == 0), stop=(cc == 15))
                nc.tensor.matmul(psO_B, v_aug[2 * m + 1][:, cc, :],
                                 eSb[:, 512:1024],
                                 start=(cc == 0), stop=(cc == 15))
                if cc == 15 and not (wq == 3 and m == 1):
                    normalize_pair(m, wq, psO_A, psO_B)
                    if m == 1 and wq < 3:
                        # group wq of BOTH pairs normalized -> finals
                        # eligible; tt 10,11 + all of g3 reserved for tail
                        tts = range(4 * wq, 4 * wq + (2 if wq == 2 else 4))
                        for tt in tts:
                            fill_q.append(finals_gen(tt))

            def pull():
                while fill_q:
                    try:
                        next(fill_q[0])
                        return True
                    except StopIteration:
                        fill_q.pop(0)
                return False

            def gen_proj_filler():
                for p in range(1, NK):
                    yield from qk_unit(False, p)     # K u1..u7
                yield from qk_unit(True, 2)
                yield from qk_unit(True, 3)
                yield from v_unit(0, 0)
                yield from v_unit(1, 0)
                yield from v_unit(0, 1)
                yield from v_unit(1, 1)
                yield from v_unit(2, 0)
                yield from v_unit(3, 0)
                yield from v_unit(2, 1)
                yield from v_unit(3, 1)
                yield from qk_unit(True, 4)
                yield from qk_unit(True, 5)
                yield from qk_unit(True, 6)
                yield from qk_unit(True, 7)

            def normalize_tail(m, wq, psO_A, psO_B):
                """Tail normalize without gpsimd: the partition scatter and
                gather run as HWDGE (sync-ring) SBUF->SBUF DMAs, avoiding
                both the gpsimd ucode-library switch (~3us) and any
                big-free-dim DVE reciprocal (iterative: ~6.5ns/elem/lane)."""
                o_sbs, rcps = [], []
                for ph, psO in ((0, psO_A), (1, psO_B)):
                    o_sb = opool.tile([65, 512], F32, tag=f"osb{ph}",
                                      bufs=2, name=f"osbT{ph}")
                    nc.vector.tensor_copy(o_sb, psO)
                    o_sbs.append(o_sb)
                    den_t = rcpool.tile([128, 4], F32, tag=f"dent{ph}",
                                        bufs=2)
                    nc.sync.dma_start(
                        out=den_t,
                        in_=o_sb[64:65, :].rearrange(
                            "a (p i) -> a p i", p=128))
                    rcp_t = rcpool.tile([128, 4], F32, tag=f"rcpt{ph}",
                                        bufs=2)
                    nc.vector.reciprocal(rcp_t, den_t)
                    flat = rcbig.tile([1, 512], F32, tag=f"rcpf{ph}", bufs=2)
                    nc.sync.dma_start(
                        out=flat[0:1, :].rearrange("a (p i) -> a p i", p=128),
                        in_=rcp_t)
                    fr = rcbig.tile([1, 512], F32R, tag=f"rr{ph}", bufs=1)
                    nc.vector.tensor_copy(fr, flat)
                    rcps.append(fr)
                for ph in range(2):
                    rc_t = psS_pool.tile([128, 1024], F32, tag="s", bufs=2,
                                         name=f"psRc{ph}")
                    rcp_bc = rc_t[0:64, 0:512]
                    nc.tensor.matmul(rcp_bc, ones64, rcps[ph],
                                     start=True, stop=True)
                    nc.vector.tensor_tensor(
                        o2T[m][64 * ph:64 * ph + 64,
                               wq * 512:(wq + 1) * 512],
                        o_sbs[ph][0:64, :], rcp_bc, mybir.AluOpType.mult)

            fill_q.append(gen_proj_filler())
            # prime the filler: K u1-u3 run inside prefix stall gaps and
            # their drains stay ~2j ahead of the scores that need them
            for _ in range(24):
                pull()

            # ---- unified j-loop.  Pair-alternating group order: since
            # projection units are pair-merged, interleaving (wq, m=0) and
            # (wq, m=1) halves the early Q/K-unit deadline pressure (Q u23
            # needed by j=32 instead of j=16, u45 by j=64, u67 by j=96). ----
            pend = []
            for j in range(128):
                wq, m, cc = j // 32, (j // 16) % 2, j % 16
                eS = emit_scores(j, wq, m, cc)
                pend.append((j, wq, m, cc, eS))
                spent = MM_NS
                if j < 40:
                    thr = 26
                else:
                    # decay 1-per-3j: repaying the early PV deferral faster
                    # than ~+0.35 drains/j overloads the PE mid-stream and
                    # the backlog resurfaces as a tail PV burst
                    thr = max(3, 26 - (j - 40) // 3)
                nd = 0
                while pend and len(pend) > thr and nd < 2:
                    emit_pv(*pend.pop(0))
                    nd += 1
                    spent += 2 * MM_NS
                if pend and pend[0][3] == 15 and nd < 3:
                    emit_pv(*pend.pop(0))
                    spent += 2 * MM_NS
                # cap pulls at true PE capacity per phase: over-pulling
                # makes the static schedule front-run the exp stream and
                # the mistimed filler block then stalls ACT on hardware
                cap = 4 if j < 14 else (3 if j < 32 else 2)
                np_ = 0
                while np_ < cap and spent < PULL_BUDGET and pull():
                    spent += MM_NS
                    np_ += 1

            # ---- tail ----
            for _ in range(len(pend)):
                emit_pv(*pend.pop(0))
            while pull():
                pass
            # gpsimd-free last normalize; the reserved finals (10,11) are
            # ready immediately and fill the PE while its chain resolves
            normalize_tail(1, 3, *state[(3, 1)])
            for tt in range(10, 16):
                for _ in finals_gen(tt, tail=True):
                    pass

    nc.compile()
    return nc


_NC_CACHE = None


def _get_program():
    global _NC_CACHE
    if _NC_CACHE is None:
        _NC_CACHE = _build_program()
    return _NC_CACHE


def _bf16(a: np.ndarray) -> np.ndarray:
    return np.asarray(a, np.float32).astype(ml_dtypes.bfloat16)


def _host_prep(inputs):
    x = np.asarray(inputs["x"], np.float32)
    wq = np.asarray(inputs["wq"], np.float32)
    wk = np.asarray(inputs["wk"], np.float32)
    wv = np.asarray(inputs["wv"], np.float32)
    wo = np.asarray(inputs["wo"], np.float32)
    bq = np.asarray(inputs["bq"], np.float32)
    bk = np.asarray(inputs["bk"], np.float32)
    bv = np.asarray(inputs["bv"], np.float32)
    rot_cos = np.asarray(inputs["rot_cos"], np.float32)
    rot_sin = np.asarray(inputs["rot_sin"], np.float32)

    cos = rot_cos[SEQ_POS]
    sin = rot_sin[SEQ_POS]

    def rope_fold_w(w):
        wv_ = w.reshape(16, 32, 2, MODEL_DIM)
        ev = wv_[:, :, 0] * cos[None, :, None] - wv_[:, :, 1] * sin[None, :, None]
        od = wv_[:, :, 0] * sin[None, :, None] + wv_[:, :, 1] * cos[None, :, None]
        return np.stack([ev, od], axis=2).reshape(MODEL_DIM, MODEL_DIM)

    def rope_fold_b(b_):
        bv_ = b_.reshape(16, 32, 2)
        ev = bv_[:, :, 0] * cos - bv_[:, :, 1] * sin
        od = bv_[:, :, 0] * sin + bv_[:, :, 1] * cos
        return np.stack([ev, od], axis=2).reshape(MODEL_DIM)

    wqT = np.ascontiguousarray(rope_fold_w(wq).T)   # [1024 xfeat, 1024 qfeat]
    wkT = np.ascontiguousarray(rope_fold_w(wk).T)
    wvT = np.ascontiguousarray(wv.T)
    bq_r = rope_fold_b(bq)
    bk_r = rope_fold_b(bk)

    # per-unit slices [p, r(xfeat%128), kc, c] -> flat [128, 1024] each
    def units(wT):
        return wT.reshape(8, 128, 8, 128).transpose(2, 1, 0, 3).reshape(
            8, 128, 1024)

    wq_u = units(wqT)
    wk_u = units(wkT)
    # wv jw halves: [128 r, 8 kc, 512 c] -> [128, 4096]
    wv_jw = [np.ascontiguousarray(
        wvT.reshape(8, 128, MODEL_DIM)[:, :, jw * 512:(jw + 1) * 512]
        .transpose(1, 0, 2)).reshape(128, 4096) for jw in range(2)]

    blob = np.concatenate(
        [wq_u[0], wq_u[1], wk_u[0], wk_u[1], wk_u[2], wk_u[3],
         wv_jw[0],
         wk_u[4], wk_u[5], wk_u[6], wk_u[7],
         wv_jw[1],
         wq_u[2], wq_u[3], wq_u[4], wq_u[5], wq_u[6], wq_u[7]],
        axis=1)
    assert blob.shape == (128, BLOB_COLS)
    blob = _bf16(blob)

    bq_sb = np.ascontiguousarray(bq_r.reshape(8, 128).T)
    bk_sb = np.ascontiguousarray(bk_r.reshape(8, 128).T)

    in_maps = []
    for cid in range(N_CORES):
        bi, g = cid // 4, cid % 4
        xTc = np.ascontiguousarray(x[bi, 512 * g:512 * (g + 1), :].T)
        xt_c = _bf16(np.ascontiguousarray(
            xTc.reshape(8, 128, RPC).transpose(1, 0, 2)))
        woTc = np.stack(
            [np.ascontiguousarray(
                wo[:, (4 * g + 2 * m) * 64:(4 * g + 2 * m + 2) * 64].T)
             for m in range(2)])
        in_maps.append({
            "xt": xt_c,
            "wblob": blob,
            "woT": _bf16(woTc),
            "bq": bq_sb, "bk": bk_sb, "bv": bv,
        })
    return in_maps, np.asarray(inputs["bo"], np.float32)


def _gather(results, bo):
    out = np.empty((B, T, MODEL_DIM), np.float32)
    for bi in range(B):
        acc = np.asarray(results[4 * bi]["outp"], np.float32)
        for g in range(1, 4):
            acc = acc + np.asarray(results[4 * bi + g]["outp"], np.float32)
        # t'' = jc*128 + r  ->  t = r*16 + jc
        acc = acc.reshape(16, 128, MODEL_DIM).transpose(1, 0, 2).reshape(
            T, MODEL_DIM)
        out[bi] = acc + bo[None, :]
    return out


def _run(inputs, trace=False, **kw):
    nc = _get_program()
    in_maps, bo = _host_prep(inputs)
    res = run_bass_kernel_spmd(nc, in_maps, list(range(N_CORES)), trace=trace,
                               **kw)
    return _gather(res.results, bo), res


def kernel(**inputs) -> np.ndarray:
    out, _ = _run(inputs)
    return out
